# revision 12
# baseline (speedup 1.0000x reference)
"""AlphaWeightedConv2d Trainium2 kernel.

Reference computation (B=32, CIN=COUT=64, H=W=112, K=3, pad=1):
    g = sigmoid(alpha[label])                     # [B, COUT]
    y = conv2d(x, W) * g[:,:,None,None] + (bias * g)[:,:,None,None]

Strategy: data-parallel over batch across 8 NeuronCores (4 samples/core).
Per core the conv is expressed as 9 shifted K=64 matmuls per output chunk
(CIN on partitions) over a row-padded image layout, so every conv tap is a
plain column offset into one SBUF tile.  Two samples ride in the two
64-partition halves of each tile; even/odd output chunks map onto the four
64x64 quadrants of the PE array (4 concurrent matmul streams, separate PSUM
banks).  The sigmoid gate is computed on host ([32,64] — negligible) and
applied by the DVE epilogue as a per-partition scale+bias while evacuating
PSUM.  x is cast to bf16 on host (harness tolerance allows it; halves input
HBM traffic); output is f32.
"""

import numpy as np
import ml_dtypes

B, CIN, COUT, H, W_SP = 32, 64, 64, 112, 112
N_CORES = 8
B_LOC = B // N_CORES          # 4 samples per core
SLOT = 114                    # padded row width (1 + 112 + 1)
NSLOT = 31                    # column slots allocated (30 rows + pad pair)
TW = NSLOT * SLOT             # 3534 tile width
R = 28                        # image rows per tile
NT = 4                        # row tiles per sample (4*28 = 112)
CH = 456                      # matmul free size: 4 row-slots * 114
CROWS = 4                     # output rows per chunk
NCHUNK = (H // CROWS)         # 28 chunks per sample pair column
TAPS = [(dy, dx) for dy in range(3) for dx in range(3)]

_cached = None


def _build():
    from concourse import bacc, tile, mybir

    bf16 = mybir.dt.bfloat16
    f32 = mybir.dt.float32
    mult = mybir.AluOpType.mult
    add = mybir.AluOpType.add
    ident = mybir.ActivationFunctionType.Identity

    nc = bacc.Bacc("TRN2", target_bir_lowering=False, debug=False,
                   num_devices=N_CORES)
    x_ext = nc.dram_tensor("x", [B_LOC * CIN, H, W_SP], bf16,
                           kind="ExternalInput")
    w_ext = nc.dram_tensor("w", [128, 9 * 64], bf16, kind="ExternalInput")
    gs_ext = nc.dram_tensor("gs", [128, 4], f32, kind="ExternalInput")
    gb_ext = nc.dram_tensor("gb", [128, 4], f32, kind="ExternalInput")
    out_ext = nc.dram_tensor("out", [B_LOC * COUT, H, W_SP], bf16,
                             kind="ExternalOutput")

    with tile.TileContext(nc) as tc:
        with (
            tc.tile_pool(name="wpool", bufs=1) as wpool,
            tc.tile_pool(name="xpool", bufs=5) as xpool,
            tc.tile_pool(name="opool", bufs=5) as opool,
            tc.tile_pool(name="pspool", bufs=6, space="PSUM") as pspool,
        ):
            w = wpool.tile([128, 9 * 64], bf16)
            gs = wpool.tile([128, 4], f32)
            gb = wpool.tile([128, 4], f32)
            nc.scalar.dma_start(w[:], w_ext.ap()[:])
            nc.scalar.dma_start(gs[:], gs_ext.ap()[:])
            nc.scalar.dma_start(gb[:], gb_ext.ap()[:])

            for p in range(2):  # sample pairs (2p, 2p+1)
                # ---- load the 4 row tiles of this pair ----
                T = []
                for t in range(NT):
                    r0 = R * t
                    xt = xpool.tile([128, TW], bf16, tag="xtile")
                    # zero the pad column pairs (cols 114k, 114k+1)
                    padv = xt[:, :].rearrange("p (k j) -> p k j", j=SLOT)
                    nc.gpsimd.memset(padv[:, :, 0:2], 0.0)
                    if t == 0:
                        nc.gpsimd.memset(xt[:, 0:SLOT + 1], 0.0)
                    if t == NT - 1:
                        nc.gpsimd.memset(xt[:, 1 + 29 * SLOT:TW], 0.0)
                    rows0 = r0 - 1 if t > 0 else 0
                    rows1 = min(r0 + R + 1, H)
                    s0 = 0 if t > 0 else 1
                    rmid = rows0 + (rows1 - rows0 + 1) // 2
                    for (ra, rb), eng in (((rows0, rmid), nc.sync),
                                          ((rmid, rows1), nc.scalar)):
                        sa = s0 + (ra - rows0)
                        dst = xt[:, 1 + sa * SLOT: 1 + (sa + rb - ra) * SLOT]
                        dst = dst.rearrange(
                            "p (s j) -> p s j", j=SLOT)[:, :, 1:113]
                        src = x_ext.ap()[p * 128:(p + 1) * 128, ra:rb, :]
                        eng.dma_start(dst, src)
                    T.append(xt)

                OSB = []
                for t in range(NT):
                    osb = opool.tile([128, R * W_SP], bf16, tag="osb")
                    OSB.append(osb)

                # ---- 28 chunks in pairs: even->psE quadrants (0,0)/(64,64),
                #      odd->psO quadrants (0,64)/(64,0) ----
                for k in range(NCHUNK // 2):
                    c0, c1 = 2 * k, 2 * k + 1
                    psE = pspool.tile([128, CH], f32, tag="ps")
                    psO = pspool.tile([128, CH], f32, tag="ps")
                    for i, (dy, dx) in enumerate(TAPS):
                        st, sp = i == 0, i == 8
                        for c, ps, swap in ((c0, psE, False), (c1, psO, True)):
                            t, u = c // 7, c % 7
                            S = 1 + (4 * u + dy) * SLOT + (dx - 1)
                            rhs = T[t]
                            aslice = ps[64:128, :] if swap else ps[0:64, :]
                            bslice = ps[0:64, :] if swap else ps[64:128, :]
                            nc.tensor.matmul(
                                aslice, w[0:64, i * 64:(i + 1) * 64],
                                rhs[0:64, S:S + CH], start=st, stop=sp)
                            nc.tensor.matmul(
                                bslice, w[64:128, i * 64:(i + 1) * 64],
                                rhs[64:128, S:S + CH], start=st, stop=sp)
                    # ---- epilogue: (psum * g) + bias*g, compact pads away;
                    #      work split between VectorE and ScalarE ----
                    for c, ps, swap in ((c0, psE, False), (c1, psO, True)):
                        t, u = c // 7, c % 7
                        ov = OSB[t][:, u * 4 * W_SP:(u + 1) * 4 * W_SP]
                        ov = ov.rearrange("p (r j) -> p r j", j=W_SP)
                        pv = ps[:, :].rearrange("p (r j) -> p r j", j=SLOT)
                        pv = pv[:, :, 1:113]
                        if not swap:
                            if k % 2 == 0:
                                nc.scalar.activation(
                                    ov, pv, ident,
                                    bias=gb[:, 2 * p:2 * p + 1],
                                    scale=gs[:, 2 * p:2 * p + 1])
                            else:
                                nc.vector.tensor_scalar(
                                    ov, pv, gs[:, 2 * p:2 * p + 1],
                                    gb[:, 2 * p:2 * p + 1], mult, add)
                        else:
                            # psO: partitions 64:128 hold sample A, 0:64 B
                            nc.vector.tensor_scalar(
                                ov[0:64], pv[64:128],
                                gs[64:128, 2 * p + 1:2 * p + 2],
                                gb[64:128, 2 * p + 1:2 * p + 2], mult, add)
                            nc.scalar.activation(
                                ov[64:128], pv[0:64], ident,
                                bias=gb[0:64, 2 * p + 1:2 * p + 2],
                                scale=gs[0:64, 2 * p + 1:2 * p + 2])
                    # ---- flush completed row tiles (split across queues) ----
                    for c in (c0, c1):
                        if c % 7 == 6:
                            t = c // 7
                            for (ra, rb), eng in (((0, R // 2), nc.sync),
                                                  ((R // 2, R), nc.scalar)):
                                dst = out_ext.ap()[p * 128:(p + 1) * 128,
                                                   R * t + ra:R * t + rb, :]
                                src = OSB[t][:, ra * W_SP:rb * W_SP]
                                src = src.rearrange(
                                    "p (r j) -> p r j", j=W_SP)
                                eng.dma_start(dst, src)

    nc.compile()
    return nc


def _prep_inputs(x, W, bias, alpha, label):
    label = np.asarray(label).astype(np.int64)
    af = np.asarray(alpha, np.float32)
    g = 1.0 / (1.0 + np.exp(-af[label]))          # [B, COUT] f32
    gbv = g * np.asarray(bias, np.float32)[None, :]

    # weights: [128, 9*64] bf16; rows 0:64 and 64:128 both = W[cout,cin,dy,dx]
    # arranged as w64[cin, tap*64 + cout]
    wf = np.asarray(W, np.float32)                # [COUT, CIN, 3, 3]
    w64 = np.transpose(wf, (1, 2, 3, 0)).reshape(CIN, 9 * COUT)
    w128 = np.concatenate([w64, w64], axis=0).astype(ml_dtypes.bfloat16)

    xb = np.asarray(x, np.float32).astype(ml_dtypes.bfloat16)
    xb = xb.reshape(B, CIN, H, W_SP)

    in_maps = []
    for core in range(N_CORES):
        s = core * B_LOC
        gsc = np.zeros((128, 4), np.float32)
        gbc = np.zeros((128, 4), np.float32)
        for p in range(2):
            a, b = s + 2 * p, s + 2 * p + 1
            gsc[0:64, 2 * p] = g[a]
            gsc[64:128, 2 * p] = g[b]
            gsc[0:64, 2 * p + 1] = g[b]      # swapped parity
            gsc[64:128, 2 * p + 1] = g[a]
            gbc[0:64, 2 * p] = gbv[a]
            gbc[64:128, 2 * p] = gbv[b]
            gbc[0:64, 2 * p + 1] = gbv[b]
            gbc[64:128, 2 * p + 1] = gbv[a]
        in_maps.append({
            "x": np.ascontiguousarray(
                xb[s:s + B_LOC].reshape(B_LOC * CIN, H, W_SP)),
            "w": w128,
            "gs": gsc,
            "gb": gbc,
        })
    return in_maps


def kernel(x, W, bias, alpha, label):
    global _cached
    from concourse.bass_utils import run_bass_kernel_spmd

    if _cached is None:
        _cached = _build()
    nc = _cached
    in_maps = _prep_inputs(x, W, bias, alpha, label)
    res = run_bass_kernel_spmd(nc, in_maps, core_ids=list(range(N_CORES)))
    out = np.concatenate(
        [np.asarray(res.results[i]["out"], np.float32).reshape(
            B_LOC, COUT, H, W_SP) for i in range(N_CORES)], axis=0)
    return out


# revision 14
# speedup vs baseline: 1.0572x; 1.0572x over previous
"""AlphaWeightedConv2d Trainium2 kernel.

Reference computation (B=32, CIN=COUT=64, H=W=112, K=3, pad=1):
    g = sigmoid(alpha[label])                     # [B, COUT]
    y = conv2d(x, W) * g[:,:,None,None] + (bias * g)[:,:,None,None]

Strategy: data-parallel over batch across 8 NeuronCores (4 samples/core).
Per core the conv is expressed as 9 shifted K=64 matmuls per output chunk
(CIN on partitions) over a row-padded image layout, so every conv tap is a
plain column offset into one SBUF tile.  Two samples ride in the two
64-partition halves of each tile; even/odd output chunks map onto the four
64x64 quadrants of the PE array (4 concurrent matmul streams, separate PSUM
banks).  The sigmoid gate is computed on host ([32,64] — negligible) and
applied by the DVE epilogue as a per-partition scale+bias while evacuating
PSUM.  x is cast to bf16 on host (harness tolerance allows it; halves input
HBM traffic); output is f32.
"""

import numpy as np
import ml_dtypes

B, CIN, COUT, H, W_SP = 32, 64, 64, 112, 112
N_CORES = 8
B_LOC = B // N_CORES          # 4 samples per core
SLOT = 114                    # padded row width (1 + 112 + 1)
NSLOT = 31                    # column slots allocated (30 rows + pad pair)
TW = NSLOT * SLOT             # 3534 tile width
R = 28                        # image rows per tile
NT = 4                        # row tiles per sample (4*28 = 112)
CH = 456                      # matmul free size: 4 row-slots * 114
CROWS = 4                     # output rows per chunk
NCHUNK = (H // CROWS)         # 28 chunks per sample pair column
TAPS = [(dy, dx) for dy in range(3) for dx in range(3)]

_cached = None


def _build():
    from concourse import bacc, tile, mybir

    bf16 = mybir.dt.bfloat16
    f32 = mybir.dt.float32
    mult = mybir.AluOpType.mult
    add = mybir.AluOpType.add
    ident = mybir.ActivationFunctionType.Identity

    nc = bacc.Bacc("TRN2", target_bir_lowering=False, debug=False,
                   num_devices=N_CORES)
    x_ext = nc.dram_tensor("x", [B_LOC * CIN, H, W_SP], bf16,
                           kind="ExternalInput")
    w_ext = nc.dram_tensor("w", [128, 9 * 64], bf16, kind="ExternalInput")
    gs_ext = nc.dram_tensor("gs", [128, 4], f32, kind="ExternalInput")
    gb_ext = nc.dram_tensor("gb", [128, 4], f32, kind="ExternalInput")
    out_ext = nc.dram_tensor("out", [B_LOC * COUT, H, W_SP], bf16,
                             kind="ExternalOutput")

    with tile.TileContext(nc) as tc:
        with (
            tc.tile_pool(name="wpool", bufs=1) as wpool,
            tc.tile_pool(name="xpool", bufs=5) as xpool,
            tc.tile_pool(name="opool", bufs=5) as opool,
            tc.tile_pool(name="pspool", bufs=6, space="PSUM") as pspool,
        ):
            w = wpool.tile([128, 9 * 64], bf16)
            gs = wpool.tile([128, 4], f32)
            gb = wpool.tile([128, 4], f32)
            nc.scalar.dma_start(w[:], w_ext.ap()[:])
            nc.scalar.dma_start(gs[:], gs_ext.ap()[:])
            nc.scalar.dma_start(gb[:], gb_ext.ap()[:])

            for p in range(2):  # sample pairs (2p, 2p+1)
                # ---- load the 4 row tiles of this pair ----
                T = []
                for t in range(NT):
                    r0 = R * t
                    xt = xpool.tile([128, TW], bf16, tag="xtile")
                    # zero the pad column pairs (cols 114k, 114k+1)
                    padv = xt[:, :].rearrange("p (k j) -> p k j", j=SLOT)
                    nc.gpsimd.memset(padv[:, :, 0:2], 0.0)
                    if t == 0:
                        nc.gpsimd.memset(xt[:, 0:SLOT + 1], 0.0)
                    if t == NT - 1:
                        nc.gpsimd.memset(xt[:, 1 + 29 * SLOT:TW], 0.0)
                    rows0 = r0 - 1 if t > 0 else 0
                    rows1 = min(r0 + R + 1, H)
                    s0 = 0 if t > 0 else 1
                    if p == 0:
                        # prologue: scalar queue is idle, split loads across
                        # both queues to cut time-to-first-matmul
                        rmid = rows0 + (rows1 - rows0 + 1) // 2
                        splits = (((rows0, rmid), nc.sync),
                                  ((rmid, rows1), nc.scalar))
                    else:
                        splits = (((rows0, rows1), nc.sync),)
                    for (ra, rb), eng in splits:
                        sa = s0 + (ra - rows0)
                        dst = xt[:, 1 + sa * SLOT: 1 + (sa + rb - ra) * SLOT]
                        dst = dst.rearrange(
                            "p (s j) -> p s j", j=SLOT)[:, :, 1:113]
                        src = x_ext.ap()[p * 128:(p + 1) * 128, ra:rb, :]
                        eng.dma_start(dst, src)
                    T.append(xt)

                OSB = []
                for t in range(NT):
                    osb = opool.tile([128, R * W_SP], bf16, tag="osb")
                    OSB.append(osb)

                # ---- 28 chunks in pairs: even->psE quadrants (0,0)/(64,64),
                #      odd->psO quadrants (0,64)/(64,0) ----
                for k in range(NCHUNK // 2):
                    c0, c1 = 2 * k, 2 * k + 1
                    psE = pspool.tile([128, CH], f32, tag="ps")
                    psO = pspool.tile([128, CH], f32, tag="ps")
                    for i, (dy, dx) in enumerate(TAPS):
                        st, sp = i == 0, i == 8
                        for c, ps, swap in ((c0, psE, False), (c1, psO, True)):
                            t, u = c // 7, c % 7
                            S = 1 + (4 * u + dy) * SLOT + (dx - 1)
                            rhs = T[t]
                            aslice = ps[64:128, :] if swap else ps[0:64, :]
                            bslice = ps[0:64, :] if swap else ps[64:128, :]
                            nc.tensor.matmul(
                                aslice, w[0:64, i * 64:(i + 1) * 64],
                                rhs[0:64, S:S + CH], start=st, stop=sp)
                            nc.tensor.matmul(
                                bslice, w[64:128, i * 64:(i + 1) * 64],
                                rhs[64:128, S:S + CH], start=st, stop=sp)
                    # ---- epilogue: (psum * g) + bias*g, compact pads away;
                    #      work split between VectorE and ScalarE ----
                    for c, ps, swap in ((c0, psE, False), (c1, psO, True)):
                        t, u = c // 7, c % 7
                        ov = OSB[t][:, u * 4 * W_SP:(u + 1) * 4 * W_SP]
                        ov = ov.rearrange("p (r j) -> p r j", j=W_SP)
                        pv = ps[:, :].rearrange("p (r j) -> p r j", j=SLOT)
                        pv = pv[:, :, 1:113]
                        if not swap:
                            if k % 2 == 0:
                                nc.scalar.activation(
                                    ov, pv, ident,
                                    bias=gb[:, 2 * p:2 * p + 1],
                                    scale=gs[:, 2 * p:2 * p + 1])
                            else:
                                nc.vector.tensor_scalar(
                                    ov, pv, gs[:, 2 * p:2 * p + 1],
                                    gb[:, 2 * p:2 * p + 1], mult, add)
                        else:
                            # psO: partitions 64:128 hold sample A, 0:64 B
                            nc.vector.tensor_scalar(
                                ov[0:64], pv[64:128],
                                gs[64:128, 2 * p + 1:2 * p + 2],
                                gb[64:128, 2 * p + 1:2 * p + 2], mult, add)
                            nc.scalar.activation(
                                ov[64:128], pv[0:64], ident,
                                bias=gb[0:64, 2 * p + 1:2 * p + 2],
                                scale=gs[0:64, 2 * p + 1:2 * p + 2])
                    # ---- flush completed row tiles ----
                    for c in (c0, c1):
                        if c % 7 == 6:
                            t = c // 7
                            dst = out_ext.ap()[p * 128:(p + 1) * 128,
                                               R * t:R * (t + 1), :]
                            src = OSB[t][:, :].rearrange(
                                "p (r j) -> p r j", j=W_SP)
                            nc.sync.dma_start(dst, src)

    nc.compile()
    return nc


def _prep_inputs(x, W, bias, alpha, label):
    label = np.asarray(label).astype(np.int64)
    af = np.asarray(alpha, np.float32)
    g = 1.0 / (1.0 + np.exp(-af[label]))          # [B, COUT] f32
    gbv = g * np.asarray(bias, np.float32)[None, :]

    # weights: [128, 9*64] bf16; rows 0:64 and 64:128 both = W[cout,cin,dy,dx]
    # arranged as w64[cin, tap*64 + cout]
    wf = np.asarray(W, np.float32)                # [COUT, CIN, 3, 3]
    w64 = np.transpose(wf, (1, 2, 3, 0)).reshape(CIN, 9 * COUT)
    w128 = np.concatenate([w64, w64], axis=0).astype(ml_dtypes.bfloat16)

    xb = np.asarray(x, np.float32).astype(ml_dtypes.bfloat16)
    xb = xb.reshape(B, CIN, H, W_SP)

    in_maps = []
    for core in range(N_CORES):
        s = core * B_LOC
        gsc = np.zeros((128, 4), np.float32)
        gbc = np.zeros((128, 4), np.float32)
        for p in range(2):
            a, b = s + 2 * p, s + 2 * p + 1
            gsc[0:64, 2 * p] = g[a]
            gsc[64:128, 2 * p] = g[b]
            gsc[0:64, 2 * p + 1] = g[b]      # swapped parity
            gsc[64:128, 2 * p + 1] = g[a]
            gbc[0:64, 2 * p] = gbv[a]
            gbc[64:128, 2 * p] = gbv[b]
            gbc[0:64, 2 * p + 1] = gbv[b]
            gbc[64:128, 2 * p + 1] = gbv[a]
        in_maps.append({
            "x": np.ascontiguousarray(
                xb[s:s + B_LOC].reshape(B_LOC * CIN, H, W_SP)),
            "w": w128,
            "gs": gsc,
            "gb": gbc,
        })
    return in_maps


def kernel(x, W, bias, alpha, label):
    global _cached
    from concourse.bass_utils import run_bass_kernel_spmd

    if _cached is None:
        _cached = _build()
    nc = _cached
    in_maps = _prep_inputs(x, W, bias, alpha, label)
    res = run_bass_kernel_spmd(nc, in_maps, core_ids=list(range(N_CORES)))
    out = np.concatenate(
        [np.asarray(res.results[i]["out"], np.float32).reshape(
            B_LOC, COUT, H, W_SP) for i in range(N_CORES)], axis=0)
    return out


# revision 19
# speedup vs baseline: 1.0732x; 1.0152x over previous
"""AlphaWeightedConv2d Trainium2 kernel.

Reference computation (B=32, CIN=COUT=64, H=W=112, K=3, pad=1):
    g = sigmoid(alpha[label])                     # [B, COUT]
    y = conv2d(x, W) * g[:,:,None,None] + (bias * g)[:,:,None,None]

Strategy: data-parallel over batch across 8 NeuronCores (4 samples/core).
Per core the conv is expressed as 9 shifted K=64 matmuls per output chunk
(CIN on partitions) over a row-padded image layout, so every conv tap is a
plain column offset into one SBUF tile.  Two samples ride in the two
64-partition halves of each tile; even/odd output chunks map onto the four
64x64 quadrants of the PE array (4 concurrent matmul streams, separate PSUM
banks).  The sigmoid gate is computed on host ([32,64] — negligible) and
applied by the DVE epilogue as a per-partition scale+bias while evacuating
PSUM.  x is cast to bf16 on host (harness tolerance allows it; halves input
HBM traffic); output is f32.
"""

import numpy as np
import ml_dtypes

B, CIN, COUT, H, W_SP = 32, 64, 64, 112, 112
N_CORES = 8
B_LOC = B // N_CORES          # 4 samples per core
SLOT = 114                    # padded row width (1 + 112 + 1)
NSLOT = 31                    # column slots allocated (30 rows + pad pair)
TW = NSLOT * SLOT             # 3534 tile width
R = 28                        # image rows per tile
NT = 4                        # row tiles per sample (4*28 = 112)
CH = 456                      # matmul free size: 4 row-slots * 114
CROWS = 4                     # output rows per chunk
NCHUNK = (H // CROWS)         # 28 chunks per sample pair column
TAPS = [(dy, dx) for dy in range(3) for dx in range(3)]

_cached = None


def _build():
    from concourse import bacc, tile, mybir

    bf16 = mybir.dt.bfloat16
    f32 = mybir.dt.float32
    mult = mybir.AluOpType.mult
    add = mybir.AluOpType.add
    ident = mybir.ActivationFunctionType.Identity

    nc = bacc.Bacc("TRN2", target_bir_lowering=False, debug=False,
                   num_devices=N_CORES)
    x_ext = nc.dram_tensor("x", [B_LOC * CIN, H, W_SP], bf16,
                           kind="ExternalInput")
    w_ext = nc.dram_tensor("w", [128, 9 * 64], bf16, kind="ExternalInput")
    gs_ext = nc.dram_tensor("gs", [128, 4], f32, kind="ExternalInput")
    gb_ext = nc.dram_tensor("gb", [128, 4], f32, kind="ExternalInput")
    out_ext = nc.dram_tensor("out", [B_LOC * COUT, H, W_SP], bf16,
                             kind="ExternalOutput")

    with tile.TileContext(nc) as tc:
        with (
            tc.tile_pool(name="wpool", bufs=1) as wpool,
            tc.tile_pool(name="xpool", bufs=5) as xpool,
            tc.tile_pool(name="opool", bufs=5) as opool,
            tc.tile_pool(name="pspool", bufs=6, space="PSUM") as pspool,
        ):
            w = wpool.tile([128, 9 * 64], bf16)
            gs = wpool.tile([128, 4], f32)
            gb = wpool.tile([128, 4], f32)
            nc.scalar.dma_start(w[:], w_ext.ap()[:])
            nc.scalar.dma_start(gs[:], gs_ext.ap()[:])
            nc.scalar.dma_start(gb[:], gb_ext.ap()[:])

            for p in range(2):  # sample pairs (2p, 2p+1)
                # ---- load the 4 row tiles of this pair ----
                T = []
                for t in range(NT):
                    r0 = R * t
                    xt = xpool.tile([128, TW], bf16, tag="xtile")
                    # zero the pad column pairs (cols 114k, 114k+1)
                    padv = xt[:, :].rearrange("p (k j) -> p k j", j=SLOT)
                    nc.gpsimd.memset(padv[:, :, 0:2], 0.0)
                    if t == 0:
                        nc.gpsimd.memset(xt[:, 0:SLOT + 1], 0.0)
                    if t == NT - 1:
                        nc.gpsimd.memset(xt[:, 1 + 29 * SLOT:TW], 0.0)
                    rows0 = r0 - 1 if t > 0 else 0
                    rows1 = min(r0 + R + 1, H)
                    s0 = 0 if t > 0 else 1
                    if p == 0 and t == 0:
                        # first tile is the critical path to the first
                        # matmul: quarter it across both queues
                        q1 = rows0 + (rows1 - rows0) // 4
                        q2 = rows0 + (rows1 - rows0) // 2
                        q3 = rows0 + 3 * (rows1 - rows0) // 4
                        splits = (((rows0, q1), nc.sync),
                                  ((q1, q2), nc.scalar),
                                  ((q2, q3), nc.sync),
                                  ((q3, rows1), nc.scalar))
                    elif p == 0:
                        # prologue: scalar queue is idle, split loads across
                        # both queues to cut time-to-first-matmul
                        rmid = rows0 + (rows1 - rows0 + 1) // 2
                        splits = (((rows0, rmid), nc.sync),
                                  ((rmid, rows1), nc.scalar))
                    else:
                        splits = (((rows0, rows1), nc.sync),)
                    for (ra, rb), eng in splits:
                        sa = s0 + (ra - rows0)
                        dst = xt[:, 1 + sa * SLOT: 1 + (sa + rb - ra) * SLOT]
                        dst = dst.rearrange(
                            "p (s j) -> p s j", j=SLOT)[:, :, 1:113]
                        src = x_ext.ap()[p * 128:(p + 1) * 128, ra:rb, :]
                        eng.dma_start(dst, src)
                    T.append(xt)

                OSB = []
                for t in range(NT):
                    osb = opool.tile([128, R * W_SP], bf16, tag="osb")
                    OSB.append(osb)

                # ---- 28 chunks in pairs: even->psE quadrants (0,0)/(64,64),
                #      odd->psO quadrants (0,64)/(64,0) ----
                for k in range(NCHUNK // 2):
                    c0, c1 = 2 * k, 2 * k + 1
                    psE = pspool.tile([128, 4 * W_SP], f32, tag="ps")
                    psO = pspool.tile([128, 4 * W_SP], f32, tag="ps")
                    for i, (dy, dx) in enumerate(TAPS):
                        st, sp = i == 0, i == 8
                        for c, ps, swap in ((c0, psE, False), (c1, psO, True)):
                            t, u = c // 7, c % 7
                            # dense 3D rhs: 4 row-slots x 112 cols, pads
                            # enter only via the slot-col window [dx, dx+112)
                            rv = T[t][:, 1:1 + 30 * SLOT].rearrange(
                                "p (s j) -> p s j", j=SLOT)
                            ra = rv[0:64, 4 * u + dy:4 * u + dy + 4,
                                    dx:dx + 112]
                            rb = rv[64:128, 4 * u + dy:4 * u + dy + 4,
                                    dx:dx + 112]
                            aslice = ps[64:128] if swap else ps[0:64]
                            bslice = ps[0:64] if swap else ps[64:128]
                            nc.tensor.matmul(
                                aslice.rearrange("p (r j) -> p r j", j=W_SP),
                                w[0:64, i * 64:(i + 1) * 64],
                                ra, start=st, stop=sp)
                            nc.tensor.matmul(
                                bslice.rearrange("p (r j) -> p r j", j=W_SP),
                                w[64:128, i * 64:(i + 1) * 64],
                                rb, start=st, stop=sp)
                    # ---- epilogue: (psum * g) + bias*g, compact pads away;
                    #      work split between VectorE and ScalarE ----
                    for c, ps, swap in ((c0, psE, False), (c1, psO, True)):
                        t, u = c // 7, c % 7
                        ov = OSB[t][:, u * 4 * W_SP:(u + 1) * 4 * W_SP]
                        pv = ps[:, :]
                        if not swap:
                            if k % 2 == 0:
                                nc.scalar.activation(
                                    ov, pv, ident,
                                    bias=gb[:, 2 * p:2 * p + 1],
                                    scale=gs[:, 2 * p:2 * p + 1])
                            else:
                                nc.vector.tensor_scalar(
                                    ov, pv, gs[:, 2 * p:2 * p + 1],
                                    gb[:, 2 * p:2 * p + 1], mult, add)
                        else:
                            # psO: partitions 64:128 hold sample A, 0:64 B
                            nc.vector.tensor_scalar(
                                ov[0:64], pv[64:128],
                                gs[64:128, 2 * p + 1:2 * p + 2],
                                gb[64:128, 2 * p + 1:2 * p + 2], mult, add)
                            nc.scalar.activation(
                                ov[64:128], pv[0:64], ident,
                                bias=gb[0:64, 2 * p + 1:2 * p + 2],
                                scale=gs[0:64, 2 * p + 1:2 * p + 2])
                    # ---- flush finished halves of row tiles ----
                    for c in (c0, c1):
                        t, u = c // 7, c % 7
                        if u == 3:
                            spans = (((0, 16), nc.sync),)
                        elif u == 6:
                            if p == 1 and t == NT - 1:
                                # last flush sits on the critical tail:
                                # split it across both queues
                                spans = (((16, 22), nc.sync),
                                         ((22, 28), nc.scalar))
                            else:
                                spans = (((16, 28), nc.sync),)
                        else:
                            continue
                        for (ra, rb), eng in spans:
                            dst = out_ext.ap()[p * 128:(p + 1) * 128,
                                               R * t + ra:R * t + rb, :]
                            src = OSB[t][:, ra * W_SP:rb * W_SP].rearrange(
                                "p (r j) -> p r j", j=W_SP)
                            eng.dma_start(dst, src)

    nc.compile()
    return nc


def _prep_inputs(x, W, bias, alpha, label):
    label = np.asarray(label).astype(np.int64)
    af = np.asarray(alpha, np.float32)
    g = 1.0 / (1.0 + np.exp(-af[label]))          # [B, COUT] f32
    gbv = g * np.asarray(bias, np.float32)[None, :]

    # weights: [128, 9*64] bf16; rows 0:64 and 64:128 both = W[cout,cin,dy,dx]
    # arranged as w64[cin, tap*64 + cout]
    wf = np.asarray(W, np.float32)                # [COUT, CIN, 3, 3]
    w64 = np.transpose(wf, (1, 2, 3, 0)).reshape(CIN, 9 * COUT)
    w128 = np.concatenate([w64, w64], axis=0).astype(ml_dtypes.bfloat16)

    xb = np.asarray(x, np.float32).astype(ml_dtypes.bfloat16)
    xb = xb.reshape(B, CIN, H, W_SP)

    in_maps = []
    for core in range(N_CORES):
        s = core * B_LOC
        gsc = np.zeros((128, 4), np.float32)
        gbc = np.zeros((128, 4), np.float32)
        for p in range(2):
            a, b = s + 2 * p, s + 2 * p + 1
            gsc[0:64, 2 * p] = g[a]
            gsc[64:128, 2 * p] = g[b]
            gsc[0:64, 2 * p + 1] = g[b]      # swapped parity
            gsc[64:128, 2 * p + 1] = g[a]
            gbc[0:64, 2 * p] = gbv[a]
            gbc[64:128, 2 * p] = gbv[b]
            gbc[0:64, 2 * p + 1] = gbv[b]
            gbc[64:128, 2 * p + 1] = gbv[a]
        in_maps.append({
            "x": np.ascontiguousarray(
                xb[s:s + B_LOC].reshape(B_LOC * CIN, H, W_SP)),
            "w": w128,
            "gs": gsc,
            "gb": gbc,
        })
    return in_maps


def kernel(x, W, bias, alpha, label):
    global _cached
    from concourse.bass_utils import run_bass_kernel_spmd

    if _cached is None:
        _cached = _build()
    nc = _cached
    in_maps = _prep_inputs(x, W, bias, alpha, label)
    res = run_bass_kernel_spmd(nc, in_maps, core_ids=list(range(N_CORES)))
    out = np.concatenate(
        [np.asarray(res.results[i]["out"], np.float32).reshape(
            B_LOC, COUT, H, W_SP) for i in range(N_CORES)], axis=0)
    return out


# revision 20
# speedup vs baseline: 1.0987x; 1.0237x over previous
"""AlphaWeightedConv2d Trainium2 kernel.

Reference computation (B=32, CIN=COUT=64, H=W=112, K=3, pad=1):
    g = sigmoid(alpha[label])                     # [B, COUT]
    y = conv2d(x, W) * g[:,:,None,None] + (bias * g)[:,:,None,None]

Strategy: data-parallel over batch across 8 NeuronCores (4 samples/core).
Per core the conv is expressed as 9 shifted K=64 matmuls per output chunk
(CIN on partitions) over a row-padded image layout, so every conv tap is a
plain column offset into one SBUF tile.  Two samples ride in the two
64-partition halves of each tile; even/odd output chunks map onto the four
64x64 quadrants of the PE array (4 concurrent matmul streams, separate PSUM
banks).  The sigmoid gate is computed on host ([32,64] — negligible) and
applied by the DVE epilogue as a per-partition scale+bias while evacuating
PSUM.  x is cast to bf16 on host (harness tolerance allows it; halves input
HBM traffic); output is f32.
"""

import numpy as np
import ml_dtypes

B, CIN, COUT, H, W_SP = 32, 64, 64, 112, 112
N_CORES = 8
B_LOC = B // N_CORES          # 4 samples per core
SLOT = 114                    # padded row width (1 + 112 + 1)
NSLOT = 31                    # column slots allocated (30 rows + pad pair)
TW = NSLOT * SLOT             # 3534 tile width
R = 28                        # image rows per tile
NT = 4                        # row tiles per sample (4*28 = 112)
CH = 456                      # matmul free size: 4 row-slots * 114
CROWS = 4                     # output rows per chunk
NCHUNK = (H // CROWS)         # 28 chunks per sample pair column
TAPS = [(dy, dx) for dy in range(3) for dx in range(3)]

_cached = None


def _build():
    from concourse import bacc, tile, mybir

    bf16 = mybir.dt.bfloat16
    f32 = mybir.dt.float32
    mult = mybir.AluOpType.mult
    add = mybir.AluOpType.add
    ident = mybir.ActivationFunctionType.Identity

    nc = bacc.Bacc("TRN2", target_bir_lowering=False, debug=False,
                   num_devices=N_CORES)
    x_ext = nc.dram_tensor("x", [B_LOC * CIN, H, W_SP], bf16,
                           kind="ExternalInput")
    w_ext = nc.dram_tensor("w", [128, 9 * 64], bf16, kind="ExternalInput")
    gs_ext = nc.dram_tensor("gs", [128, 4], f32, kind="ExternalInput")
    gb_ext = nc.dram_tensor("gb", [128, 4], f32, kind="ExternalInput")
    out_ext = nc.dram_tensor("out", [B_LOC * COUT, H, W_SP], bf16,
                             kind="ExternalOutput")

    with tile.TileContext(nc) as tc:
        with (
            tc.tile_pool(name="wpool", bufs=1) as wpool,
            tc.tile_pool(name="xpool", bufs=8) as xpool,
            tc.tile_pool(name="opool", bufs=5) as opool,
            tc.tile_pool(name="pspool", bufs=8, space="PSUM") as pspool,
        ):
            w = wpool.tile([128, 9 * 64], bf16)
            gs = wpool.tile([128, 4], f32)
            gb = wpool.tile([128, 4], f32)
            nc.scalar.dma_start(w[:], w_ext.ap()[:])
            nc.scalar.dma_start(gs[:], gs_ext.ap()[:])
            nc.scalar.dma_start(gb[:], gb_ext.ap()[:])

            for p in range(2):  # sample pairs (2p, 2p+1)
                # ---- load the 4 row tiles of this pair ----
                T = []
                for t in range(NT):
                    r0 = R * t
                    xt = xpool.tile([128, TW], bf16, tag="xtile")
                    # zero the pad column pairs (cols 114k, 114k+1)
                    padv = xt[:, :].rearrange("p (k j) -> p k j", j=SLOT)
                    nc.gpsimd.memset(padv[:, :, 0:2], 0.0)
                    if t == 0:
                        nc.gpsimd.memset(xt[:, 0:SLOT + 1], 0.0)
                    if t == NT - 1:
                        nc.gpsimd.memset(xt[:, 1 + 29 * SLOT:TW], 0.0)
                    rows0 = r0 - 1 if t > 0 else 0
                    rows1 = min(r0 + R + 1, H)
                    s0 = 0 if t > 0 else 1
                    if p == 0 and t == 0:
                        # first tile is the critical path to the first
                        # matmul: quarter it across both queues
                        q1 = rows0 + (rows1 - rows0) // 4
                        q2 = rows0 + (rows1 - rows0) // 2
                        q3 = rows0 + 3 * (rows1 - rows0) // 4
                        splits = (((rows0, q1), nc.sync),
                                  ((q1, q2), nc.scalar),
                                  ((q2, q3), nc.sync),
                                  ((q3, rows1), nc.scalar))
                    elif p == 0:
                        # prologue: scalar queue is idle, split loads across
                        # both queues to cut time-to-first-matmul
                        rmid = rows0 + (rows1 - rows0 + 1) // 2
                        splits = (((rows0, rmid), nc.sync),
                                  ((rmid, rows1), nc.scalar))
                    else:
                        splits = (((rows0, rows1), nc.sync),)
                    for (ra, rb), eng in splits:
                        sa = s0 + (ra - rows0)
                        dst = xt[:, 1 + sa * SLOT: 1 + (sa + rb - ra) * SLOT]
                        dst = dst.rearrange(
                            "p (s j) -> p s j", j=SLOT)[:, :, 1:113]
                        src = x_ext.ap()[p * 128:(p + 1) * 128, ra:rb, :]
                        eng.dma_start(dst, src)
                    T.append(xt)

                OSB = []
                for t in range(NT):
                    osb = opool.tile([128, R * W_SP], bf16, tag="osb")
                    OSB.append(osb)

                # ---- 28 chunks in pairs: even->psE quadrants (0,0)/(64,64),
                #      odd->psO quadrants (0,64)/(64,0) ----
                for k in range(NCHUNK // 2):
                    c0, c1 = 2 * k, 2 * k + 1
                    psE = pspool.tile([128, 4 * W_SP], f32, tag="ps")
                    psO = pspool.tile([128, 4 * W_SP], f32, tag="ps")
                    for i, (dy, dx) in enumerate(TAPS):
                        st, sp = i == 0, i == 8
                        for c, ps, swap in ((c0, psE, False), (c1, psO, True)):
                            t, u = c // 7, c % 7
                            # dense 3D rhs: 4 row-slots x 112 cols, pads
                            # enter only via the slot-col window [dx, dx+112)
                            rv = T[t][:, 1:1 + 30 * SLOT].rearrange(
                                "p (s j) -> p s j", j=SLOT)
                            ra = rv[0:64, 4 * u + dy:4 * u + dy + 4,
                                    dx:dx + 112]
                            rb = rv[64:128, 4 * u + dy:4 * u + dy + 4,
                                    dx:dx + 112]
                            aslice = ps[64:128] if swap else ps[0:64]
                            bslice = ps[0:64] if swap else ps[64:128]
                            nc.tensor.matmul(
                                aslice.rearrange("p (r j) -> p r j", j=W_SP),
                                w[0:64, i * 64:(i + 1) * 64],
                                ra, start=st, stop=sp)
                            nc.tensor.matmul(
                                bslice.rearrange("p (r j) -> p r j", j=W_SP),
                                w[64:128, i * 64:(i + 1) * 64],
                                rb, start=st, stop=sp)
                    # ---- epilogue: (psum * g) + bias*g, compact pads away;
                    #      work split between VectorE and ScalarE ----
                    for c, ps, swap in ((c0, psE, False), (c1, psO, True)):
                        t, u = c // 7, c % 7
                        ov = OSB[t][:, u * 4 * W_SP:(u + 1) * 4 * W_SP]
                        pv = ps[:, :]
                        if not swap:
                            if k % 2 == 0:
                                nc.scalar.activation(
                                    ov, pv, ident,
                                    bias=gb[:, 2 * p:2 * p + 1],
                                    scale=gs[:, 2 * p:2 * p + 1])
                            else:
                                nc.vector.tensor_scalar(
                                    ov, pv, gs[:, 2 * p:2 * p + 1],
                                    gb[:, 2 * p:2 * p + 1], mult, add)
                        else:
                            # psO: partitions 64:128 hold sample A, 0:64 B
                            nc.vector.tensor_scalar(
                                ov[0:64], pv[64:128],
                                gs[64:128, 2 * p + 1:2 * p + 2],
                                gb[64:128, 2 * p + 1:2 * p + 2], mult, add)
                            nc.scalar.activation(
                                ov[64:128], pv[0:64], ident,
                                bias=gb[0:64, 2 * p + 1:2 * p + 2],
                                scale=gs[0:64, 2 * p + 1:2 * p + 2])
                    # ---- flush finished halves of row tiles ----
                    for c in (c0, c1):
                        t, u = c // 7, c % 7
                        if u == 3:
                            spans = (((0, 16), nc.sync),)
                        elif u == 6:
                            if p == 1 and t == NT - 1:
                                # last flush sits on the critical tail:
                                # split it across both queues
                                spans = (((16, 22), nc.sync),
                                         ((22, 28), nc.scalar))
                            else:
                                spans = (((16, 28), nc.sync),)
                        else:
                            continue
                        for (ra, rb), eng in spans:
                            dst = out_ext.ap()[p * 128:(p + 1) * 128,
                                               R * t + ra:R * t + rb, :]
                            src = OSB[t][:, ra * W_SP:rb * W_SP].rearrange(
                                "p (r j) -> p r j", j=W_SP)
                            eng.dma_start(dst, src)

    nc.compile()
    return nc


def _prep_inputs(x, W, bias, alpha, label):
    label = np.asarray(label).astype(np.int64)
    af = np.asarray(alpha, np.float32)
    g = 1.0 / (1.0 + np.exp(-af[label]))          # [B, COUT] f32
    gbv = g * np.asarray(bias, np.float32)[None, :]

    # weights: [128, 9*64] bf16; rows 0:64 and 64:128 both = W[cout,cin,dy,dx]
    # arranged as w64[cin, tap*64 + cout]
    wf = np.asarray(W, np.float32)                # [COUT, CIN, 3, 3]
    w64 = np.transpose(wf, (1, 2, 3, 0)).reshape(CIN, 9 * COUT)
    w128 = np.concatenate([w64, w64], axis=0).astype(ml_dtypes.bfloat16)

    xb = np.asarray(x, np.float32).astype(ml_dtypes.bfloat16)
    xb = xb.reshape(B, CIN, H, W_SP)

    in_maps = []
    for core in range(N_CORES):
        s = core * B_LOC
        gsc = np.zeros((128, 4), np.float32)
        gbc = np.zeros((128, 4), np.float32)
        for p in range(2):
            a, b = s + 2 * p, s + 2 * p + 1
            gsc[0:64, 2 * p] = g[a]
            gsc[64:128, 2 * p] = g[b]
            gsc[0:64, 2 * p + 1] = g[b]      # swapped parity
            gsc[64:128, 2 * p + 1] = g[a]
            gbc[0:64, 2 * p] = gbv[a]
            gbc[64:128, 2 * p] = gbv[b]
            gbc[0:64, 2 * p + 1] = gbv[b]
            gbc[64:128, 2 * p + 1] = gbv[a]
        in_maps.append({
            "x": np.ascontiguousarray(
                xb[s:s + B_LOC].reshape(B_LOC * CIN, H, W_SP)),
            "w": w128,
            "gs": gsc,
            "gb": gbc,
        })
    return in_maps


def kernel(x, W, bias, alpha, label):
    global _cached
    from concourse.bass_utils import run_bass_kernel_spmd

    if _cached is None:
        _cached = _build()
    nc = _cached
    in_maps = _prep_inputs(x, W, bias, alpha, label)
    res = run_bass_kernel_spmd(nc, in_maps, core_ids=list(range(N_CORES)))
    out = np.concatenate(
        [np.asarray(res.results[i]["out"], np.float32).reshape(
            B_LOC, COUT, H, W_SP) for i in range(N_CORES)], axis=0)
    return out
